# revision 12
# baseline (speedup 1.0000x reference)
"""NT-Xent contrastive loss kernel for 8 Trainium2 NeuronCores.

Reference computation (N=8192, D=512, tau=0.5):
    zl = l2norm_rows(left); zr = l2norm_rows(right)
    refl    = exp(zl @ zl.T / tau)
    between = exp(zl @ zr.T / tau)
    denom   = refl.sum(1) + between.sum(1) - diag(refl)
    loss    = -log(diag(between) / denom)

Fused per-row form used here (diag(refl) == exp(1/tau) == e^2 exactly since
rows of zl are unit-norm):
    loss[m] = log( S_l[m] + S_r[m] - e^2 ) - 2 * (zl_m . zr_m)
with S_x[m] = sum_n exp(2 * zl_m . zx_n).  The NxN similarity matrices are
never materialized: each [128, 512] PSUM tile of similarities is consumed by
one ScalarE exp-activation whose accum_out produces the partial row sum.

Sharding: data-parallel over rows. Core c owns rows [c*1024, (c+1)*1024).
Each core receives its own row-blocks of left/right in f32 (natural layout,
for row norms + the between-diagonal at full precision) and the full
transposed left/right in bf16 (K-major layout for the matmuls).  Column
normalization happens on device: sum-of-squares via a ones-vector matmul,
sqrt (ScalarE) + reciprocal (VectorE), broadcast to 128 partitions via a
K=1 ones matmul, then one VectorE multiply producing the resident
normalized bf16 tensors.  No cross-core communication is needed.
"""

import numpy as np
import ml_dtypes
from contextlib import ExitStack

import concourse.bass as bass
import concourse.tile as tile
from concourse import bacc, mybir
from concourse.bass import ds, ts
from concourse.bass_utils import run_bass_kernel_spmd
from concourse.masks import make_identity

P = 128          # partitions
D = 512          # feature dim
N = 8192         # rows
NCORES = 8
BLK = N // NCORES          # 1024 rows per core
KCH = D // P               # 4 k-chunks of 128
MT = BLK // P              # 8 m-tiles per core
NT = N // 512              # 16 n-tiles of 512 per tensor
NJ = N // 1024             # 8 dma tiles of 1024 per tensor
E2 = float(np.exp(2.0))    # exp(1/tau * ||zl_m||^2) = e^2

F32 = mybir.dt.float32
BF16 = mybir.dt.bfloat16
AF = mybir.ActivationFunctionType
OP = mybir.AluOpType

_CACHE = {}


def _body(ctx, tc, lblk, rblk, lT, rT, loss_out, n=N, blk=BLK):
    nc = tc.nc
    mt = blk // P
    nt = n // 512
    nj = n // 1024

    const_pool = ctx.enter_context(tc.tile_pool(name="const", bufs=1))
    persist = ctx.enter_context(tc.tile_pool(name="persist", bufs=1))
    blk_pool = ctx.enter_context(tc.tile_pool(name="blk", bufs=2))
    small = ctx.enter_context(tc.tile_pool(name="small", bufs=4))
    raw_pool = ctx.enter_context(tc.tile_pool(name="raw", bufs=2))
    sq_pool = ctx.enter_context(tc.tile_pool(name="sq", bufs=2))
    row_pool = ctx.enter_context(tc.tile_pool(name="row", bufs=2))
    bcast_pool = ctx.enter_context(tc.tile_pool(name="bcast", bufs=2))
    exp_pool = ctx.enter_context(tc.tile_pool(name="exps", bufs=4))
    rs_pool = ctx.enter_context(tc.tile_pool(name="rs", bufs=2))

    psum_mm = ctx.enter_context(tc.tile_pool(name="psmm", bufs=4, space="PSUM"))
    psum_ssq = ctx.enter_context(tc.tile_pool(name="psq", bufs=2, space="PSUM"))
    psum_tr = ctx.enter_context(tc.tile_pool(name="pstr", bufs=2, space="PSUM"))

    # constants
    ones_col = const_pool.tile([P, 1], BF16, tag="ones_col")
    nc.gpsimd.memset(ones_col[:], 1.0)
    ones_row = const_pool.tile([1, P], BF16, tag="ones_row")
    nc.gpsimd.memset(ones_row[:], 1.0)
    ident = const_pool.tile([P, P], BF16, tag="ident")
    make_identity(nc, ident[:])
    neg_e2 = const_pool.tile([P, 1], F32, tag="neg_e2")
    nc.gpsimd.memset(neg_e2[:], -E2)

    # persistent tensors
    znT_l = persist.tile([P, KCH, n], BF16, tag="znT_l")
    znT_r = persist.tile([P, KCH, n], BF16, tag="znT_r")
    lhsT_norm = persist.tile([P, KCH, blk], BF16, tag="lhsT")
    bd = persist.tile([P, mt], F32, tag="bd")          # zl_m . zr_m (normalized)
    loss_sb = persist.tile([P, mt], F32, tag="loss_sb")

    # ---------------- Phase 0: own row block (f32, natural layout) ----------
    # row norms of own block, between-diagonal, and the normalized-transposed
    # lhsT for the matmuls.
    for t in range(mt):
        lt = blk_pool.tile([P, D], F32, tag="lt")
        nc.sync.dma_start(lt[:], lblk[ts(t, P), :])
        rt = blk_pool.tile([P, D], F32, tag="rt")
        nc.sync.dma_start(rt[:], rblk[ts(t, P), :])

        scr = blk_pool.tile([P, D], F32, tag="scr")
        ssql = small.tile([P, 1], F32, tag="ssql")
        nc.vector.tensor_mul(scr[:], lt[:], lt[:])
        nc.vector.tensor_reduce(ssql[:], scr[:], axis=mybir.AxisListType.X, op=OP.add)
        ssqr = small.tile([P, 1], F32, tag="ssqr")
        nc.vector.tensor_mul(scr[:], rt[:], rt[:])
        nc.vector.tensor_reduce(ssqr[:], scr[:], axis=mybir.AxisListType.X, op=OP.add)
        dotv = small.tile([P, 1], F32, tag="dotv")
        nc.vector.tensor_mul(scr[:], lt[:], rt[:])
        nc.vector.tensor_reduce(dotv[:], scr[:], axis=mybir.AxisListType.X, op=OP.add)

        nl_ = small.tile([P, 1], F32, tag="nl")
        nc.scalar.activation(nl_[:], ssql[:], AF.Sqrt)
        invl = small.tile([P, 1], F32, tag="invl")
        nc.vector.reciprocal(invl[:], nl_[:])
        nr_ = small.tile([P, 1], F32, tag="nr")
        nc.scalar.activation(nr_[:], ssqr[:], AF.Sqrt)
        invr = small.tile([P, 1], F32, tag="invr")
        nc.vector.reciprocal(invr[:], nr_[:])

        # between-diagonal: dot * invl * invr   (full f32 precision)
        nc.vector.tensor_scalar(
            out=bd[:, ts(t, 1)], in0=dotv[:], scalar1=invl[:], scalar2=invr[:],
            op0=OP.mult, op1=OP.mult,
        )

        # normalized own rows -> bf16, then TensorE-transpose into lhsT layout
        natn = blk_pool.tile([P, D], BF16, tag="natn")
        nc.vector.tensor_scalar_mul(natn[:], lt[:], invl[:])
        for k in range(KCH):
            tp = psum_tr.tile([P, P], BF16, tag="ps_shared")
            nc.tensor.transpose(tp[:], natn[:, ts(k, P)], ident[:])
            nc.vector.tensor_copy(lhsT_norm[:, k, ts(t, P)], tp[:])

    # ---------------- Phase 1: stream full tensors, normalize columns -------
    for znT, src in ((znT_l, lT), (znT_r, rT)):
        for j in range(nj):
            raw = raw_pool.tile([P, KCH, 1024], BF16, tag="raw")
            for k in range(KCH):
                nc.sync.dma_start(raw[:, k, :], src[k, :, ds(j * 1024, 1024)])
            for h in range(2):
                nvis = j * 1024 + h * 512
                sqps = psum_ssq.tile([1, 512], F32, tag="ssqp")
                for k in range(KCH):
                    sq = sq_pool.tile([P, 512], BF16, tag="sq")
                    nc.vector.tensor_mul(
                        sq[:], raw[:, k, ds(h * 512, 512)], raw[:, k, ds(h * 512, 512)]
                    )
                    nc.tensor.matmul(
                        sqps[:], ones_col[:], sq[:], start=(k == 0), stop=(k == KCH - 1)
                    )
                nrow = row_pool.tile([1, 512], F32, tag="nrow")
                nc.scalar.activation(nrow[:], sqps[:], AF.Sqrt)
                irow = row_pool.tile([1, 512], F32, tag="irow")
                nc.vector.reciprocal(irow[:], nrow[:])
                irow_bf = row_pool.tile([1, 512], BF16, tag="irowbf")
                nc.vector.tensor_copy(irow_bf[:], irow[:])
                bps = psum_tr.tile([P, 512], F32, tag="ps_shared")
                nc.tensor.matmul(bps[:], ones_row[:], irow_bf[:], start=True, stop=True)
                bc = bcast_pool.tile([P, 512], BF16, tag="bc")
                nc.vector.tensor_copy(bc[:], bps[:])
                for k in range(KCH):
                    nc.vector.tensor_mul(
                        znT[:, k, ds(nvis, 512)], raw[:, k, ds(h * 512, 512)], bc[:]
                    )

    # ---------------- Phase 2: similarity matmuls + fused exp/row-sum -------
    for t in range(mt):
        rowsums = rs_pool.tile([P, 2 * nt], F32, tag="rowsums")
        for xi, znT in enumerate((znT_l, znT_r)):
            for ni in range(nt):
                ps = psum_mm.tile([P, 512], F32, tag="mm")
                for k in range(KCH):
                    nc.tensor.matmul(
                        ps[:],
                        lhsT_norm[:, k, ts(t, P)],
                        znT[:, k, ts(ni, 512)],
                        start=(k == 0),
                        stop=(k == KCH - 1),
                    )
                ex = exp_pool.tile([P, 512], BF16, tag="ex")
                nc.scalar.activation(
                    ex[:], ps[:], AF.Exp, scale=2.0,
                    accum_out=rowsums[:, ds(xi * nt + ni, 1)],
                )
        s_t = small.tile([P, 1], F32, tag="s_t")
        nc.vector.tensor_reduce(s_t[:], rowsums[:], axis=mybir.AxisListType.X, op=OP.add)
        logd = small.tile([P, 1], F32, tag="logd")
        nc.scalar.activation(logd[:], s_t[:], AF.Ln, bias=neg_e2[:])
        nc.vector.scalar_tensor_tensor(
            out=loss_sb[:, ts(t, 1)], in0=bd[:, ts(t, 1)], scalar=-2.0, in1=logd[:],
            op0=OP.mult, op1=OP.add,
        )

    nc.sync.dma_start(loss_out[:, :], loss_sb[:])


def _build(n=N, blk=BLK):
    mt = blk // P
    nc = bacc.Bacc("TRN2", target_bir_lowering=False, debug=False, num_devices=NCORES)
    lblk = nc.dram_tensor("lblk", [blk, D], F32, kind="ExternalInput").ap()
    rblk = nc.dram_tensor("rblk", [blk, D], F32, kind="ExternalInput").ap()
    lT = nc.dram_tensor("lT", [KCH, P, n], BF16, kind="ExternalInput").ap()
    rT = nc.dram_tensor("rT", [KCH, P, n], BF16, kind="ExternalInput").ap()
    loss = nc.dram_tensor("loss", [P, mt], F32, kind="ExternalOutput").ap()
    with tile.TileContext(nc) as tc, ExitStack() as ctx:
        _body(ctx, tc, lblk, rblk, lT, rT, loss, n=n, blk=blk)
    nc.compile()
    return nc


def _get_nc():
    if "nc" not in _CACHE:
        _CACHE["nc"] = _build()
    return _CACHE["nc"]


def _in_maps(left, right):
    left = np.asarray(left, dtype=np.float32)
    right = np.asarray(right, dtype=np.float32)
    bf = ml_dtypes.bfloat16
    lT = np.ascontiguousarray(left.T).astype(bf).reshape(KCH, P, N)
    rT = np.ascontiguousarray(right.T).astype(bf).reshape(KCH, P, N)
    maps = []
    for c in range(NCORES):
        maps.append({
            "lblk": np.ascontiguousarray(left[c * BLK:(c + 1) * BLK]),
            "rblk": np.ascontiguousarray(right[c * BLK:(c + 1) * BLK]),
            "lT": lT,
            "rT": rT,
        })
    return maps


def _gather(results):
    # loss dram tile is [128 partitions, 8 m-tiles]; row m = t*128 + p
    parts = [np.asarray(r["loss"]).T.reshape(-1) for r in results]
    return np.concatenate(parts).astype(np.float32)


def run_traced(left, right):
    """Run with NTFF profiling; returns (loss, exec_time_ns)."""
    res = run_bass_kernel_spmd(
        _get_nc(), _in_maps(left, right), list(range(NCORES)), trace=True
    )
    return _gather(res.results), res.exec_time_ns


def kernel(left, right):
    res = run_bass_kernel_spmd(
        _get_nc(), _in_maps(left, right), list(range(NCORES))
    )
    return _gather(res.results)


# revision 13
# speedup vs baseline: 3.2212x; 3.2212x over previous
"""NT-Xent contrastive loss kernel for 8 Trainium2 NeuronCores.

Reference computation (N=8192, D=512, tau=0.5):
    zl = l2norm_rows(left); zr = l2norm_rows(right)
    refl    = exp(zl @ zl.T / tau)
    between = exp(zl @ zr.T / tau)
    denom   = refl.sum(1) + between.sum(1) - diag(refl)
    loss    = -log(diag(between) / denom)

Fused per-row form used here (diag(refl) == exp(1/tau) == e^2 exactly since
rows of zl are unit-norm):
    loss[m] = log( S_l[m] + S_r[m] - e^2 ) - 2 * (zl_m . zr_m)
with S_x[m] = sum_n exp(2 * zl_m . zx_n).  The NxN similarity matrices are
never materialized: each [128, 512] PSUM tile of similarities is consumed by
one ScalarE exp-activation whose accum_out produces the partial row sum.

Sharding: data-parallel over rows. Core c owns rows [c*1024, (c+1)*1024).
Each core receives its own row-blocks of left/right in f32 (natural layout,
for row norms + the between-diagonal at full precision) and the full
transposed left/right in bf16 (K-major layout for the matmuls).  Column
normalization happens on device: sum-of-squares via a ones-vector matmul,
sqrt (ScalarE) + reciprocal (VectorE), broadcast to 128 partitions via a
K=1 ones matmul, then one VectorE multiply producing the resident
normalized bf16 tensors.  No cross-core communication is needed.
"""

import numpy as np
import ml_dtypes
from contextlib import ExitStack

import concourse.bass as bass
import concourse.tile as tile
from concourse import bacc, mybir
from concourse.bass import ds, ts
from concourse.bass_utils import run_bass_kernel_spmd
from concourse.masks import make_identity

P = 128          # partitions
D = 512          # feature dim
N = 8192         # rows
NCORES = 8
BLK = N // NCORES          # 1024 rows per core
KCH = D // P               # 4 k-chunks of 128
MT = BLK // P              # 8 m-tiles per core
NT = N // 512              # 16 n-tiles of 512 per tensor
NJ = N // 1024             # 8 dma tiles of 1024 per tensor
E2 = float(np.exp(2.0))    # exp(1/tau * ||zl_m||^2) = e^2

F32 = mybir.dt.float32
BF16 = mybir.dt.bfloat16
AF = mybir.ActivationFunctionType
OP = mybir.AluOpType

_CACHE = {}


def _body(ctx, tc, lblk, rblk, lT, rT, loss_out, n=N, blk=BLK):
    nc = tc.nc
    mt = blk // P
    nt = n // 512
    nj = n // 1024

    const_pool = ctx.enter_context(tc.tile_pool(name="const", bufs=1))
    persist = ctx.enter_context(tc.tile_pool(name="persist", bufs=1))
    blk_pool = ctx.enter_context(tc.tile_pool(name="blk", bufs=2))
    small = ctx.enter_context(tc.tile_pool(name="small", bufs=4))
    raw_pool = ctx.enter_context(tc.tile_pool(name="raw", bufs=2))
    sq_pool = ctx.enter_context(tc.tile_pool(name="sq", bufs=2))
    row_pool = ctx.enter_context(tc.tile_pool(name="row", bufs=2))
    bcast_pool = ctx.enter_context(tc.tile_pool(name="bcast", bufs=2))
    exp_pool = ctx.enter_context(tc.tile_pool(name="exps", bufs=4))
    rs_pool = ctx.enter_context(tc.tile_pool(name="rs", bufs=2))

    psum_mm = ctx.enter_context(tc.tile_pool(name="psmm", bufs=4, space="PSUM"))
    psum_ssq = ctx.enter_context(tc.tile_pool(name="psq", bufs=2, space="PSUM"))
    psum_tr = ctx.enter_context(tc.tile_pool(name="pstr", bufs=2, space="PSUM"))

    # constants
    ones_col = const_pool.tile([P, 1], BF16, tag="ones_col")
    nc.gpsimd.memset(ones_col[:], 1.0)
    ones_row = const_pool.tile([1, P], BF16, tag="ones_row")
    nc.gpsimd.memset(ones_row[:], 1.0)
    ident = const_pool.tile([P, P], BF16, tag="ident")
    make_identity(nc, ident[:])
    neg_e2 = const_pool.tile([P, 1], F32, tag="neg_e2")
    nc.gpsimd.memset(neg_e2[:], -E2)

    # persistent tensors
    znT_l = persist.tile([P, KCH, n], BF16, tag="znT_l")
    znT_r = persist.tile([P, KCH, n], BF16, tag="znT_r")
    lhsT_norm = persist.tile([P, KCH, blk], BF16, tag="lhsT")
    bd = persist.tile([P, mt], F32, tag="bd")          # zl_m . zr_m (normalized)
    loss_sb = persist.tile([P, mt], F32, tag="loss_sb")

    # ---------------- Phase 0: own row block (f32, natural layout) ----------
    # row norms of own block, between-diagonal, and the normalized-transposed
    # lhsT for the matmuls.
    for t in range(mt):
        lt = blk_pool.tile([P, D], F32, tag="lt")
        nc.sync.dma_start(lt[:], lblk[ts(t, P), :])
        rt = blk_pool.tile([P, D], F32, tag="rt")
        nc.sync.dma_start(rt[:], rblk[ts(t, P), :])

        scr = blk_pool.tile([P, D], F32, tag="scr")
        ssql = small.tile([P, 1], F32, tag="ssql")
        nc.vector.tensor_mul(scr[:], lt[:], lt[:])
        nc.vector.tensor_reduce(ssql[:], scr[:], axis=mybir.AxisListType.X, op=OP.add)
        ssqr = small.tile([P, 1], F32, tag="ssqr")
        nc.vector.tensor_mul(scr[:], rt[:], rt[:])
        nc.vector.tensor_reduce(ssqr[:], scr[:], axis=mybir.AxisListType.X, op=OP.add)
        dotv = small.tile([P, 1], F32, tag="dotv")
        nc.vector.tensor_mul(scr[:], lt[:], rt[:])
        nc.vector.tensor_reduce(dotv[:], scr[:], axis=mybir.AxisListType.X, op=OP.add)

        nl_ = small.tile([P, 1], F32, tag="nl")
        nc.scalar.activation(nl_[:], ssql[:], AF.Sqrt)
        invl = small.tile([P, 1], F32, tag="invl")
        nc.vector.reciprocal(invl[:], nl_[:])
        nr_ = small.tile([P, 1], F32, tag="nr")
        nc.scalar.activation(nr_[:], ssqr[:], AF.Sqrt)
        invr = small.tile([P, 1], F32, tag="invr")
        nc.vector.reciprocal(invr[:], nr_[:])

        # between-diagonal: dot * invl * invr   (full f32 precision)
        nc.vector.tensor_scalar(
            out=bd[:, ts(t, 1)], in0=dotv[:], scalar1=invl[:], scalar2=invr[:],
            op0=OP.mult, op1=OP.mult,
        )

        # normalized own rows -> bf16, then TensorE-transpose into lhsT layout
        natn = blk_pool.tile([P, D], BF16, tag="natn")
        nc.vector.tensor_scalar_mul(natn[:], lt[:], invl[:])
        for k in range(KCH):
            tp = psum_tr.tile([P, P], BF16, tag="ps_shared")
            nc.tensor.transpose(tp[:], natn[:, ts(k, P)], ident[:])
            nc.vector.tensor_copy(lhsT_norm[:, k, ts(t, P)], tp[:])

    # ---------------- Phase 1: stream full tensors, normalize columns -------
    for znT, src in ((znT_l, lT), (znT_r, rT)):
        for j in range(nj):
            raw = raw_pool.tile([P, KCH, 1024], BF16, tag="raw")
            for k in range(KCH):
                nc.sync.dma_start(raw[:, k, :], src[k, :, ds(j * 1024, 1024)])
            for h in range(2):
                nvis = j * 1024 + h * 512
                sqps = psum_ssq.tile([1, 512], F32, tag="ssqp")
                for k in range(KCH):
                    sq = sq_pool.tile([P, 512], BF16, tag="sq")
                    nc.vector.tensor_mul(
                        sq[:], raw[:, k, ds(h * 512, 512)], raw[:, k, ds(h * 512, 512)]
                    )
                    nc.tensor.matmul(
                        sqps[:], ones_col[:], sq[:], start=(k == 0), stop=(k == KCH - 1)
                    )
                nrow = row_pool.tile([1, 512], F32, tag="nrow")
                nc.scalar.activation(nrow[:], sqps[:], AF.Sqrt)
                irow = row_pool.tile([1, 512], F32, tag="irow")
                nc.vector.reciprocal(irow[:], nrow[:])
                irow_bf = row_pool.tile([1, 512], BF16, tag="irowbf")
                nc.vector.tensor_copy(irow_bf[:], irow[:])
                bps = psum_tr.tile([P, 512], F32, tag="ps_shared")
                nc.tensor.matmul(bps[:], ones_row[:], irow_bf[:], start=True, stop=True)
                bc = bcast_pool.tile([P, 512], BF16, tag="bc")
                nc.vector.tensor_copy(bc[:], bps[:])
                for k in range(KCH):
                    nc.vector.tensor_mul(
                        znT[:, k, ds(nvis, 512)], raw[:, k, ds(h * 512, 512)], bc[:]
                    )

    # ---------------- Phase 2: similarity matmuls + fused exp/row-sum -------
    s_all = persist.tile([P, mt], F32, tag="s_all")
    for t in range(mt):
        rowsums = rs_pool.tile([P, 2 * nt], F32, tag="rowsums")
        for xi, znT in enumerate((znT_l, znT_r)):
            for ni in range(nt):
                ps = psum_mm.tile([P, 512], F32, tag="mm")
                for k in range(KCH):
                    nc.tensor.matmul(
                        ps[:],
                        lhsT_norm[:, k, ts(t, P)],
                        znT[:, k, ts(ni, 512)],
                        start=(k == 0),
                        stop=(k == KCH - 1),
                    )
                ex = exp_pool.tile([P, 512], BF16, tag="ex")
                nc.scalar.activation(
                    ex[:], ps[:], AF.Exp, scale=2.0,
                    accum_out=rowsums[:, ds(xi * nt + ni, 1)],
                )
        nc.vector.tensor_reduce(
            s_all[:, ts(t, 1)], rowsums[:], axis=mybir.AxisListType.X, op=OP.add
        )

    # ---------------- Phase 3: loss epilogue (one Ln, no table thrash) ------
    logd = small.tile([P, mt], F32, tag="logd")
    nc.scalar.activation(logd[:], s_all[:], AF.Ln, bias=neg_e2[:])
    nc.vector.scalar_tensor_tensor(
        out=loss_sb[:], in0=bd[:], scalar=-2.0, in1=logd[:],
        op0=OP.mult, op1=OP.add,
    )
    nc.sync.dma_start(loss_out[:, :], loss_sb[:])


def _build(n=N, blk=BLK):
    mt = blk // P
    nc = bacc.Bacc("TRN2", target_bir_lowering=False, debug=False, num_devices=NCORES)
    lblk = nc.dram_tensor("lblk", [blk, D], F32, kind="ExternalInput").ap()
    rblk = nc.dram_tensor("rblk", [blk, D], F32, kind="ExternalInput").ap()
    lT = nc.dram_tensor("lT", [KCH, P, n], BF16, kind="ExternalInput").ap()
    rT = nc.dram_tensor("rT", [KCH, P, n], BF16, kind="ExternalInput").ap()
    loss = nc.dram_tensor("loss", [P, mt], F32, kind="ExternalOutput").ap()
    with tile.TileContext(nc) as tc, ExitStack() as ctx:
        _body(ctx, tc, lblk, rblk, lT, rT, loss, n=n, blk=blk)
    nc.compile()
    return nc


def _get_nc():
    if "nc" not in _CACHE:
        _CACHE["nc"] = _build()
    return _CACHE["nc"]


def _in_maps(left, right):
    left = np.asarray(left, dtype=np.float32)
    right = np.asarray(right, dtype=np.float32)
    bf = ml_dtypes.bfloat16
    lT = np.ascontiguousarray(left.T).astype(bf).reshape(KCH, P, N)
    rT = np.ascontiguousarray(right.T).astype(bf).reshape(KCH, P, N)
    maps = []
    for c in range(NCORES):
        maps.append({
            "lblk": np.ascontiguousarray(left[c * BLK:(c + 1) * BLK]),
            "rblk": np.ascontiguousarray(right[c * BLK:(c + 1) * BLK]),
            "lT": lT,
            "rT": rT,
        })
    return maps


def _gather(results):
    # loss dram tile is [128 partitions, 8 m-tiles]; row m = t*128 + p
    parts = [np.asarray(r["loss"]).T.reshape(-1) for r in results]
    return np.concatenate(parts).astype(np.float32)


def run_traced(left, right):
    """Run with NTFF profiling; returns (loss, exec_time_ns)."""
    res = run_bass_kernel_spmd(
        _get_nc(), _in_maps(left, right), list(range(NCORES)), trace=True
    )
    return _gather(res.results), res.exec_time_ns


def kernel(left, right):
    res = run_bass_kernel_spmd(
        _get_nc(), _in_maps(left, right), list(range(NCORES))
    )
    return _gather(res.results)
